# revision 41
# baseline (speedup 1.0000x reference)
"""GQA forward (B=2,N=2048,D=2048,H=32,KV=8,DH=64, causal) on 8 trn2 cores.

Sharding: 2-way data parallel over batch x 4-way tensor parallel over heads
(each core: 8 q-heads = 2 kv-heads, keeping group structure). Row-parallel
out-proj; the all-reduce over the 4 TP shards (+ bias) happens on host at
gather time.

v2 design (vs baseline three serial phases):
  - all matmul operands bf16 (fp32 PSUM accumulation) -> FWL weight loads,
    half DMA/SBUF traffic, no fp32r narrow-moving penalty.
  - one fused pipeline: projection of q-block b+1 and out-projection of
    q-block b-1 are interleaved as PE filler between the attention matmuls
    of q-block b, so the tensor engine never idles long enough for the HAM
    clock gate to re-throttle to 1.2 GHz (the baseline spent 389us at half
    clock during attention).
  - scores for the 2 kv-heads of a head-pair run concurrently in PE row
    groups (K=64 contractions at base partitions 0 / 64).
  - causal mask applied by accumulating an identity-matmul of a -30000
    constant onto the diagonal score blocks (no DVE in the exp->ctx path).
  - exp batched: one ACT instruction per [128, 1024] PSUM span (both heads
    of a pair for one key block).
  - V projected directly in [tokens, dh] orientation with xs chunks as the
    stationary operand (no PE transposes).
"""
import os
import sys

import numpy as np

if "/opt/trn_rl_repo" not in sys.path:
    sys.path.insert(0, "/opt/trn_rl_repo")

import ml_dtypes

import concourse.bacc as bacc
import concourse.tile as tile
from concourse import mybir
from concourse.bass_utils import run_bass_kernel_spmd
from concourse.masks import make_identity

F32 = mybir.dt.float32
F32R = mybir.dt.float32r
BF16 = mybir.dt.bfloat16
EXP = mybir.ActivationFunctionType.Exp
LN = mybir.ActivationFunctionType.Ln

B, N, D = 2, 2048, 2048
H, KV, DH = 32, 8, 64
G = H // KV                      # 4 q-heads per kv head
HPC, KVPC = 8, 2                 # heads / kv-heads per core
DQ = HPC * DH                    # 512 per-core q projection width
NBW = 512                        # q-block width
NB = N // NBW                    # 4 q-blocks
DC = D // 128                    # 16 contraction chunks
NEG = -30000.0                   # causal mask additive constant

_CACHED = {}


def _build():
    nc = bacc.Bacc("TRN2", target_bir_lowering=False, debug=False,
                   num_devices=8)

    # Both Exp and Ln are used (softmax exp; 1/x as exp(-ln x)). The table
    # picker takes the first set containing each function, which would
    # alternate exp_and_others <-> natural_log at ~2.7us per switch. Trim
    # the cached table map (values only -- keys/order define act_func_set
    # ids and must stay) so the only set advertising Exp/Ln is the combined
    # one; it is then loaded exactly once.
    from concourse.hw_specs import get_activation_tables
    tabs = get_activation_tables(nc.m.arch)
    comb = tabs.get("natural_log_exp_and_others")
    if comb and EXP in comb and LN in comb:
        for name, fns in tabs.items():
            if name != "natural_log_exp_and_others":
                fns.discard(EXP)
                fns.discard(LN)

    xT = nc.dram_tensor("xT", [D, N], BF16, kind="ExternalInput")
    Wq = nc.dram_tensor("Wq", [D, DQ], BF16, kind="ExternalInput")
    Wk = nc.dram_tensor("Wk", [D, KVPC * DH], BF16, kind="ExternalInput")
    Wv = nc.dram_tensor("Wv", [D, KVPC * DH], BF16, kind="ExternalInput")
    Wo = nc.dram_tensor("Wo", [DQ, D], BF16, kind="ExternalInput")
    OUT = nc.dram_tensor("out", [N, D], F32, kind="ExternalOutput")

    scale = 1.0 / np.sqrt(DH)

    with tile.TileContext(nc) as tc:
        with (
            tc.tile_pool(name="persist", bufs=1) as pp,
            tc.tile_pool(name="wq", bufs=16) as wqp,
            tc.tile_pool(name="wkv", bufs=32) as wkvp,
            tc.tile_pool(name="wo", bufs=16) as wop,
            tc.tile_pool(name="xs", bufs=16) as xsp,
            tc.tile_pool(name="psb", bufs=4) as psbp,
            tc.tile_pool(name="outs", bufs=2) as outp,
            tc.tile_pool(name="small", bufs=2) as smp,
            tc.tile_pool(name="sc_ps", bufs=2, space="PSUM") as scp,
            tc.tile_pool(name="c_ps", bufs=2, space="PSUM") as cpp,
            tc.tile_pool(name="pj_ps", bufs=1, space="PSUM") as pjp,
            tc.tile_pool(name="ms_ps", bufs=1, space="PSUM") as msp,
        ):
            # ---- persistent sbuf state ----
            identf = pp.tile([128, 128], F32, tag="identf")
            make_identity(nc, identf[:])
            ident = pp.tile([128, 128], BF16, tag="ident")
            nc.vector.tensor_copy(ident[:], identf[:])

            # additive causal mask for a 128x128 diagonal block:
            # mask[k, j] = 0 if j >= k else NEG (local query j, local key k)
            mknf = pp.tile([128, 128], F32, tag="mknf")
            nc.gpsimd.memset(mknf[:], 0.0)
            nc.gpsimd.affine_select(
                out=mknf[:], in_=mknf[:],
                compare_op=mybir.AluOpType.is_ge,
                fill=NEG, base=0,
                pattern=[[1, 128]],
                channel_multiplier=-1,
            )
            maskneg = pp.tile([128, 128], BF16, tag="maskneg")
            nc.vector.tensor_copy(maskneg[:], mknf[:])

            onesf = pp.tile([128, 64], F32, tag="onesf")
            nc.vector.memset(onesf[:], 1.0)
            ones_b = pp.tile([128, 64], BF16, tag="onesb")
            nc.vector.tensor_copy(ones_b[:], onesf[:])
            ones_r = pp.tile([128, 64], F32R, tag="onesr")
            nc.vector.tensor_copy(ones_r[:], onesf[:])

            qt = [pp.tile([128, N], BF16, tag=f"qt{g}", name=f"qt{g}")
                  for g in range(4)]
            kt = pp.tile([128, N], BF16, tag="kt")
            # va[m]: [ A_dh(0:64) | onesA(64) | B_dh(65:129) | onesB(129) ]
            va = [pp.tile([128, 2 * (DH + 1)], BF16, tag=f"va{m}",
                          name=f"va{m}") for m in range(N // 128)]
            for m in range(N // 128):
                nc.vector.tensor_copy(va[m][:, DH:DH + 1], ones_b[:, 0:1])
                nc.vector.tensor_copy(va[m][:, 2 * DH + 1:2 * DH + 2],
                                      ones_b[:, 0:1])
            ctxT = [pp.tile([128, N], BF16, tag=f"ct{g}", name=f"ct{g}")
                    for g in range(4)]

            # ---- weights ----
            wq_sb, wk_sb, wv_sb = [], [], []
            for dc in range(DC):
                t = wqp.tile([128, DQ], BF16, tag="w")
                nc.scalar.dma_start(out=t[:], in_=Wq[dc * 128:(dc + 1) * 128, :])
                wq_sb.append(t)
            for dc in range(DC):
                t = wkvp.tile([128, KVPC * DH], BF16, tag="wk")
                nc.scalar.dma_start(out=t[:], in_=Wk[dc * 128:(dc + 1) * 128, :])
                wk_sb.append(t)
            for dc in range(DC):
                t = wkvp.tile([128, KVPC * DH], BF16, tag="wv")
                nc.scalar.dma_start(out=t[:], in_=Wv[dc * 128:(dc + 1) * 128, :])
                wv_sb.append(t)
            # wo rides the idle gpsimd queue: it is not needed until the
            # first out-projection (stage 1), and on sync it would delay
            # the xs streams.
            wo_sb = {}
            for j in range(4):
                for ob in range(4):
                    t = wop.tile([128, NBW], BF16, tag="wo")
                    nc.gpsimd.dma_start(
                        out=t[:],
                        in_=Wo[j * 128:(j + 1) * 128, ob * NBW:(ob + 1) * NBW])
                    wo_sb[(j, ob)] = t

            # ---- filler machinery ----------------------------------------
            # Thunks emitting PE-heavy work between attention ops so the
            # tensor engine never starves while ACT runs exp. hi = next
            # block's projections (must drain before that block's attention);
            # lo = previous block's out-projection (can carry over stages).
            # Queues hold (pe_cost_ns, fn). Emission is budget-paced: during
            # attention the ACT exp stream is the pacer, so filler is
            # released only up to the ACT-minus-attention-PE slack --
            # anything more would delay later scores and starve ACT;
            # anything less idles the PE and risks a HAM re-throttle.
            filler_hi = []
            filler_lo = []
            clock = {"act": 0.0, "pe": 0.0}

            # Keep a ~2.5us PE backlog on top of the ACT pacing: the HAM
            # clock gate only returns to full speed after a ~3.4us window of
            # CONTIGUOUS PE activity, so an exactly-paced schedule (PE ~80%
            # busy with frequent sub-us waits) that ever throttles stays at
            # half clock forever. A standing backlog keeps the PE densely
            # busy and recoverable.
            def emit_filler_budget(allow_lo=False):
                while filler_hi or (allow_lo and filler_lo):
                    q = filler_hi if filler_hi else filler_lo
                    cost, fn = q[0][0], q[0][1]
                    if clock["act"] + 2500.0 - clock["pe"] < cost:
                        break
                    q.pop(0)
                    clock["pe"] += cost
                    fn()

            # Alternate filler psum allocations between the two filler banks
            # so a unit's copy-out overlaps the next unit's matmuls instead
            # of write-after-read blocking on a single bank.
            fill_ctr = [0]

            def fill_ps_tile():
                fill_ctr[0] += 1
                if fill_ctr[0] % 2:
                    return msp.tile([128, NBW], F32, tag="ms", name="fps")
                return pjp.tile([128, NBW], F32, tag="pj", name="fps")

            def drain_hi():
                while filler_hi:
                    cost, fn, _key = filler_hi.pop(0)
                    clock["pe"] += cost
                    fn()

            def force_hi(b, level):
                # Pull queued projection work this attention point depends
                # on. Thunks are keyed (block, level); the hi queue is FIFO
                # and level-ordered within a block, so front-popping
                # suffices.
                while filler_hi:
                    key = filler_hi[0][2]
                    if key is None or key[0] != b or key[1] > level:
                        break
                    cost, fn, _key = filler_hi.pop(0)
                    clock["pe"] += cost
                    fn()

            xs_tiles = {}          # b -> list of 16 sbuf tiles

            def emit_xs_dma(b):
                ts = []
                for dc in range(DC):
                    t = xsp.tile([128, NBW], BF16, tag="xs", name="xs")
                    nc.sync.dma_start(
                        out=t[:],
                        in_=xT[dc * 128:(dc + 1) * 128,
                               b * NBW:(b + 1) * NBW])
                    ts.append(t)
                xs_tiles[b] = ts

            def proj_thunks(b):
                """Projection of q-block b: 6 bank-passes over resident xs,
                each pass split into 4-dc chunks (~850ns PE). Pass order
                [q0, k, v, q1, q2, q3] matches the gate order attention(b)
                pulls them in: pair g needs q-slab g; the k/v of block b are
                first needed at key-block 4b (the diagonal region)."""
                thunks = []

                def qk_chunk(g, c0, cell):
                    # g in 0..3 -> q slab g ; g == 4 -> k
                    def mk():
                        xs = xs_tiles[b]
                        if c0 == 0:
                            cell.append(fill_ps_tile())
                        ps = cell[0]
                        for dc in range(c0, c0 + 4):
                            stat = (wq_sb[dc][:, g * 128:(g + 1) * 128]
                                    if g < 4 else wk_sb[dc][:])
                            nc.tensor.matmul(ps[:], stat, xs[dc][:],
                                             start=(dc == 0),
                                             stop=(dc == DC - 1))
                        if c0 + 4 == DC:
                            dst = qt[g] if g < 4 else kt
                            nc.vector.tensor_copy(
                                dst[:, b * NBW:(b + 1) * NBW], ps[:])
                    return mk

                def v_chunk(c0, cell):
                    def mk():
                        xs = xs_tiles[b]
                        if c0 == 0:
                            cell.append(fill_ps_tile())
                        ps = cell[0]
                        for dc in range(c0, c0 + 4):
                            for i in range(4):
                                nc.tensor.matmul(
                                    ps[:, i * 128:(i + 1) * 128],
                                    xs[dc][:, i * 128:(i + 1) * 128],
                                    wv_sb[dc][:],
                                    start=(dc == 0 and i == 0),
                                    stop=(dc == DC - 1 and i == 3),
                                    skip_group_check=True)
                        if c0 + 4 == DC:
                            for i in range(4):
                                m = b * 4 + i
                                nc.vector.tensor_copy(
                                    va[m][:, 0:DH],
                                    ps[:, i * 128:i * 128 + 64])
                                nc.vector.tensor_copy(
                                    va[m][:, DH + 1:2 * DH + 1],
                                    ps[:, i * 128 + 64:i * 128 + 128])
                    return mk

                def qk_pass(g, level):
                    cell = []
                    return [(1010, qk_chunk(g, c0, cell), (b, level))
                            for c0 in range(0, DC, 4)]

                thunks += qk_pass(0, 0)
                thunks += qk_pass(4, 1)
                cell = []
                thunks += [(1490, v_chunk(c0, cell), (b, 1))
                           for c0 in range(0, DC, 4)]
                for g in range(1, 4):
                    thunks += qk_pass(g, g + 1)
                return thunks

            def outproj_thunks(b):
                """Out-projection of q-block b (4 row tiles)."""
                thunks = []

                def ob_unit(nt, ob, osb):
                    def mk():
                        ops = fill_ps_tile()
                        for j in range(4):
                            nc.tensor.matmul(
                                ops[:],
                                ctxT[j][:, nt * 128:(nt + 1) * 128],
                                wo_sb[(j, ob)][:],
                                start=(j == 0), stop=(j == 3))
                        nc.vector.tensor_copy(
                            osb[0][:, ob * NBW:(ob + 1) * NBW], ops[:])
                    return mk

                def out_dma(nt, osb):
                    def mk():
                        nc.gpsimd.dma_start(
                            out=OUT[nt * 128:(nt + 1) * 128, :], in_=osb[0][:])
                    return mk

                for i in range(4):
                    nt = b * 4 + i
                    osb = []

                    def alloc(osb=osb):
                        osb.append(outp.tile([128, D], F32, tag="osb",
                                             name="osb"))
                    thunks.append((0, alloc, None))
                    for ob in range(4):
                        thunks.append((1010, ob_unit(nt, ob, osb), None))
                    thunks.append((0, out_dma(nt, osb), None))
                return thunks

            # ---- norm: ctx / softmax-denominator, into ctxT ---------------
            def emit_pair_norm(c_a, c_b, g, q0):
                # Critical prefix: 4 plain copies move the ctx rows and the
                # denominator rows (psum partition 64, from the ones column
                # of va) out of PSUM so the two c banks free immediately.
                # Everything after runs async: broadcast both denominators
                # into one psum bank (col groups 0/64), 1/x as exp(-ln x)
                # on ACT (same table set as the attention exp; DVE
                # reciprocal costs ~3.3us per call), then normalize. Head
                # B's rows reach partitions 64:128 via a sbuf-to-sbuf DMA
                # (engines cannot shift partitions).
                lrowA = smp.tile([65, NBW], F32R, tag="lrA", name="lrA")
                nc.vector.tensor_copy(lrowA[DH:DH + 1, :], c_a[DH:DH + 1, :])
                lrowB = smp.tile([65, NBW], F32R, tag="lrB", name="lrB")
                nc.vector.tensor_copy(lrowB[DH:DH + 1, :], c_b[DH:DH + 1, :])
                cuA = smp.tile([DH, NBW], F32, tag="cuA", name="cuA")
                nc.vector.tensor_copy(cuA[:], c_a[0:DH, :])
                cuB = smp.tile([DH, NBW], F32, tag="cuB", name="cuB")
                nc.vector.tensor_copy(cuB[:], c_b[0:DH, :])
                for lrow, cu, par in ((lrowA, cuA, 0), (lrowB, cuB, 1)):
                    rb_ps = msp.tile([DH, NBW], F32, tag="ms", name="rbps")
                    nc.tensor.matmul(rb_ps[:], ones_r[DH:DH + 1, 0:DH],
                                     lrow[DH:DH + 1, :], start=True, stop=True)
                    lg = smp.tile([DH, NBW], F32, tag=f"lg{par}", name="lg")
                    nc.scalar.activation(lg[:], rb_ps[:], LN)
                    rb = smp.tile([DH, NBW], F32, tag=f"rb{par}", name="rb")
                    nc.scalar.activation(rb[:], lg[:], EXP, scale=-1.0)
                    if par == 0:
                        nc.vector.tensor_mul(ctxT[g][0:DH, q0:q0 + NBW],
                                             cu[:], rb[:])
                    else:
                        tmp = smp.tile([DH, NBW], BF16, tag="ctmp",
                                       name="ctmp")
                        nc.vector.tensor_mul(tmp[:], cu[:], rb[:])
                        nc.sync.dma_start(
                            out=ctxT[g][DH:2 * DH, q0:q0 + NBW], in_=tmp[:])

            # ---- attention for one q-block, with filler interleave --------
            def attention(b):
                q0 = b * NBW
                n_kb = 4 * b + 4
                for g in range(4):
                    force_hi(b, 0 if g == 0 else g + 1)
                    c_a = cpp.tile([DH + 1, NBW], F32, tag="c", name="ca")
                    c_b = cpp.tile([DH + 1, NBW], F32, tag="c", name="cb")
                    pend = None
                    for kb in range(n_kb):
                        m0 = kb * 128
                        diag = kb >= 4 * b
                        if diag:
                            force_hi(b, 1)
                        off = max(0, m0 - q0)
                        T = scp.tile([128, 2 * NBW], F32, tag="sc", name="T")
                        # scores: kv0 rows 0:64 / kv1 rows 64:128 run as
                        # concurrent PE row-groups. Head A only needs cols
                        # >= off (earlier cols belong to fully-masked
                        # queries); head B keeps full width so the exp span
                        # [off : 2*NBW] reads initialized psum only.
                        nc.tensor.matmul(T[:, off:NBW],
                                         kt[0:64, m0:m0 + 128],
                                         qt[g][0:64, q0 + off:q0 + NBW],
                                         start=True, stop=not diag,
                                         skip_group_check=True)
                        nc.tensor.matmul(T[:, NBW:2 * NBW],
                                         kt[64:128, m0:m0 + 128],
                                         qt[g][64:128, q0:q0 + NBW],
                                         start=True, stop=not diag,
                                         skip_group_check=True)
                        if diag:
                            nc.tensor.matmul(T[:, off:off + 128],
                                             ident[:], maskneg[:],
                                             start=False, stop=True,
                                             skip_group_check=True)
                            nc.tensor.matmul(T[:, NBW + off:NBW + off + 128],
                                             ident[:], maskneg[:],
                                             start=False, stop=True,
                                             skip_group_check=True)
                        p = psbp.tile([128, 2 * NBW], BF16, tag="p", name="p")
                        nc.scalar.activation(p[:, off:2 * NBW],
                                             T[:, off:2 * NBW], EXP,
                                             scale=float(scale))
                        w = NBW - off
                        clock["act"] += (2 * NBW - off + 172) / 1.2 + 120
                        clock["pe"] += 3 * (w / 2.4 + 45)
                        if diag:
                            clock["pe"] += 2 * (128 / 2.4 + 45)
                        if pend is not None:
                            pend()
                        emit_filler_budget(allow_lo=(b == NB - 1))

                        def _ctx(kb=kb, off=off, p=p, c_a=c_a, c_b=c_b):
                            nc.tensor.matmul(
                                c_a[:, off:NBW],
                                va[kb][:, 0:DH + 1],
                                p[:, off:NBW],
                                start=(kb == 0), stop=(kb == n_kb - 1),
                                skip_group_check=True)
                            nc.tensor.matmul(
                                c_b[:, off:NBW],
                                va[kb][:, DH + 1:2 * DH + 2],
                                p[:, NBW + off:2 * NBW],
                                start=(kb == 0), stop=(kb == n_kb - 1),
                                skip_group_check=True)
                        pend = _ctx
                    pend()
                    emit_pair_norm(c_a, c_b, g, q0)
                    clock["act"] += 2 * ((NBW + 172) / 1.2 + 120)
                    clock["pe"] += 2 * (NBW / 2.4 + 45)
                    emit_filler_budget(allow_lo=(b == NB - 1))

            # ---- main schedule -------------------------------------------
            emit_xs_dma(0)
            p0 = proj_thunks(0)
            for _cost, th, _key in p0[:12]:   # q0, k, v passes up front
                th()
            filler_hi.extend(p0[12:])         # q1..q3 pulled by pair gates
            emit_xs_dma(1)
            for b in range(NB):
                if b + 1 < NB:
                    filler_hi.extend(proj_thunks(b + 1))
                if b + 2 < NB:
                    filler_hi.append(
                        (0, lambda b=b: emit_xs_dma(b + 2), (b + 2, -1)))
                if b >= 1:
                    filler_lo.extend(outproj_thunks(b - 1))
                attention(b)
            drain_hi()
            # Tail: flush leftover lo first (its ctxT inputs are ready), then
            # the last block's out-projection; bank alternation keeps the PE
            # dense so the HAM gate can recover full clock.
            for _cost, fn, _key in filler_lo:
                fn()
            for _cost, fn, _key in outproj_thunks(NB - 1):
                fn()

    nc.compile()
    return nc


def kernel(x, Wq, Wk, Wv, Wo, bo):
    x = np.asarray(x, dtype=np.float32)
    Wq = np.asarray(Wq, dtype=np.float32)
    Wk = np.asarray(Wk, dtype=np.float32)
    Wv = np.asarray(Wv, dtype=np.float32)
    Wo = np.asarray(Wo, dtype=np.float32)
    bo = np.asarray(bo, dtype=np.float32)

    if "nc" not in _CACHED:
        _CACHED["nc"] = _build()
    nc = _CACHED["nc"]

    bf = ml_dtypes.bfloat16
    in_maps = []
    for c in range(8):
        b, t = c // 4, c % 4
        xTc = np.ascontiguousarray(x[b].T).astype(bf)
        # q slab g holds [kv-head 2t head g | kv-head 2t+1 head g]
        qcols = []
        for g in range(4):
            for kvl in range(KVPC):
                h = (2 * t + kvl) * G + g
                qcols.append(Wq[:, h * DH:(h + 1) * DH])
        wq_c = np.ascontiguousarray(np.concatenate(qcols, axis=1)).astype(bf)
        wk_c = np.ascontiguousarray(Wk[:, t * 128:(t + 1) * 128]).astype(bf)
        wv_c = np.ascontiguousarray(Wv[:, t * 128:(t + 1) * 128]).astype(bf)
        # Wo rows must follow the ctxT head-pair layout: slab g holds
        # [head (kv 2t, g) | head (kv 2t+1, g)]
        wrows = []
        for g in range(4):
            for kvl in range(KVPC):
                h = (2 * t + kvl) * G + g
                wrows.append(Wo[h * DH:(h + 1) * DH, :])
        wo_c = np.ascontiguousarray(np.concatenate(wrows, axis=0)).astype(bf)
        in_maps.append({"xT": xTc, "Wq": wq_c, "Wk": wk_c, "Wv": wv_c,
                        "Wo": wo_c})

    trace = bool(int(os.environ.get("GQA_TRACE", "0")))
    kwargs = {}
    if trace:
        import tempfile
        td = os.environ.get("GQA_TRACE_DIR") or tempfile.mkdtemp(prefix="gqa_")
        kwargs = dict(trace=True, tmpdir=td)
    res = run_bass_kernel_spmd(nc, in_maps, list(range(8)), **kwargs)
    _CACHED["last_result"] = res

    out = np.empty((B, N, D), dtype=np.float32)
    for b in range(B):
        acc = res.results[4 * b]["out"].astype(np.float32)
        for t in range(1, 4):
            acc = acc + res.results[4 * b + t]["out"]
        out[b] = acc + bo[None, :]
    return out


# revision 44
# speedup vs baseline: 1.0999x; 1.0999x over previous
"""GQA forward (B=2,N=2048,D=2048,H=32,KV=8,DH=64, causal) on 8 trn2 cores.

Sharding: 2-way data parallel over batch x 4-way tensor parallel over heads
(each core: 8 q-heads = 2 kv-heads, keeping group structure). Row-parallel
out-proj; the all-reduce over the 4 TP shards (+ bias) happens on host at
gather time.

v2 design (vs baseline three serial phases):
  - all matmul operands bf16 (fp32 PSUM accumulation) -> FWL weight loads,
    half DMA/SBUF traffic, no fp32r narrow-moving penalty.
  - one fused pipeline: projection of q-block b+1 and out-projection of
    q-block b-1 are interleaved as PE filler between the attention matmuls
    of q-block b, so the tensor engine never idles long enough for the HAM
    clock gate to re-throttle to 1.2 GHz (the baseline spent 389us at half
    clock during attention).
  - scores for the 2 kv-heads of a head-pair run concurrently in PE row
    groups (K=64 contractions at base partitions 0 / 64).
  - causal mask applied by accumulating an identity-matmul of a -30000
    constant onto the diagonal score blocks (no DVE in the exp->ctx path).
  - exp batched: one ACT instruction per [128, 1024] PSUM span (both heads
    of a pair for one key block).
  - V projected directly in [tokens, dh] orientation with xs chunks as the
    stationary operand (no PE transposes).
"""
import os
import sys

import numpy as np

if "/opt/trn_rl_repo" not in sys.path:
    sys.path.insert(0, "/opt/trn_rl_repo")

import ml_dtypes

import concourse.bacc as bacc
import concourse.tile as tile
from concourse import mybir
from concourse.bass_utils import run_bass_kernel_spmd
from concourse.masks import make_identity

F32 = mybir.dt.float32
F32R = mybir.dt.float32r
BF16 = mybir.dt.bfloat16
EXP = mybir.ActivationFunctionType.Exp
LN = mybir.ActivationFunctionType.Ln

B, N, D = 2, 2048, 2048
H, KV, DH = 32, 8, 64
G = H // KV                      # 4 q-heads per kv head
HPC, KVPC = 8, 2                 # heads / kv-heads per core
DQ = HPC * DH                    # 512 per-core q projection width
NBW = 512                        # q-block width
NB = N // NBW                    # 4 q-blocks
DC = D // 128                    # 16 contraction chunks
NEG = -30000.0                   # causal mask additive constant

_CACHED = {}


def _build():
    nc = bacc.Bacc("TRN2", target_bir_lowering=False, debug=False,
                   num_devices=8)

    # Both Exp and Ln are used (softmax exp; 1/x as exp(-ln x)). The table
    # picker takes the first set containing each function, which would
    # alternate exp_and_others <-> natural_log at ~2.7us per switch. Trim
    # the cached table map (values only -- keys/order define act_func_set
    # ids and must stay) so the only set advertising Exp/Ln is the combined
    # one; it is then loaded exactly once.
    from concourse.hw_specs import get_activation_tables
    tabs = get_activation_tables(nc.m.arch)
    comb = tabs.get("natural_log_exp_and_others")
    if comb and EXP in comb and LN in comb:
        for name, fns in tabs.items():
            if name != "natural_log_exp_and_others":
                fns.discard(EXP)
                fns.discard(LN)

    xT = nc.dram_tensor("xT", [D, N], BF16, kind="ExternalInput")
    Wq = nc.dram_tensor("Wq", [D, DQ], BF16, kind="ExternalInput")
    Wk = nc.dram_tensor("Wk", [D, KVPC * DH], BF16, kind="ExternalInput")
    Wv = nc.dram_tensor("Wv", [D, KVPC * DH], BF16, kind="ExternalInput")
    Wo = nc.dram_tensor("Wo", [DQ, D], BF16, kind="ExternalInput")
    OUT = nc.dram_tensor("out", [N, D], F32, kind="ExternalOutput")

    scale = 1.0 / np.sqrt(DH)

    with tile.TileContext(nc) as tc:
        with (
            tc.tile_pool(name="persist", bufs=1) as pp,
            tc.tile_pool(name="wq", bufs=16) as wqp,
            tc.tile_pool(name="wkv", bufs=32) as wkvp,
            tc.tile_pool(name="wo", bufs=16) as wop,
            tc.tile_pool(name="xs", bufs=16) as xsp,
            tc.tile_pool(name="psb", bufs=4) as psbp,
            tc.tile_pool(name="outs", bufs=2) as outp,
            tc.tile_pool(name="small", bufs=2) as smp,
            tc.tile_pool(name="sc_ps", bufs=2, space="PSUM") as scp,
            tc.tile_pool(name="c_ps", bufs=2, space="PSUM") as cpp,
            tc.tile_pool(name="pj_ps", bufs=1, space="PSUM") as pjp,
            tc.tile_pool(name="ms_ps", bufs=1, space="PSUM") as msp,
        ):
            # ---- persistent sbuf state ----
            identf = pp.tile([128, 128], F32, tag="identf")
            make_identity(nc, identf[:])
            ident = pp.tile([128, 128], BF16, tag="ident")
            nc.vector.tensor_copy(ident[:], identf[:])

            # additive causal mask for a 128x128 diagonal block:
            # mask[k, j] = 0 if j >= k else NEG (local query j, local key k)
            mknf = pp.tile([128, 128], F32, tag="mknf")
            nc.gpsimd.memset(mknf[:], 0.0)
            nc.gpsimd.affine_select(
                out=mknf[:], in_=mknf[:],
                compare_op=mybir.AluOpType.is_ge,
                fill=NEG, base=0,
                pattern=[[1, 128]],
                channel_multiplier=-1,
            )
            maskneg = pp.tile([128, 128], BF16, tag="maskneg")
            nc.vector.tensor_copy(maskneg[:], mknf[:])

            onesf = pp.tile([128, 64], F32, tag="onesf")
            nc.vector.memset(onesf[:], 1.0)
            ones_b = pp.tile([128, 64], BF16, tag="onesb")
            nc.vector.tensor_copy(ones_b[:], onesf[:])
            ones_r = pp.tile([128, 64], F32R, tag="onesr")
            nc.vector.tensor_copy(ones_r[:], onesf[:])

            qt = [pp.tile([128, N], BF16, tag=f"qt{g}", name=f"qt{g}")
                  for g in range(4)]
            kt = pp.tile([128, N], BF16, tag="kt")
            # va[m]: [ A_dh(0:64) | onesA(64) | B_dh(65:129) | onesB(129) ]
            va = [pp.tile([128, 2 * (DH + 1)], BF16, tag=f"va{m}",
                          name=f"va{m}") for m in range(N // 128)]
            for m in range(N // 128):
                nc.vector.tensor_copy(va[m][:, DH:DH + 1], ones_b[:, 0:1])
                nc.vector.tensor_copy(va[m][:, 2 * DH + 1:2 * DH + 2],
                                      ones_b[:, 0:1])
            ctxT = [pp.tile([128, N], BF16, tag=f"ct{g}", name=f"ct{g}")
                    for g in range(4)]

            # ---- weights ----
            wq_sb, wk_sb, wv_sb = [], [], []
            for dc in range(DC):
                t = wqp.tile([128, DQ], BF16, tag="w")
                nc.scalar.dma_start(out=t[:], in_=Wq[dc * 128:(dc + 1) * 128, :])
                wq_sb.append(t)
            for dc in range(DC):
                t = wkvp.tile([128, KVPC * DH], BF16, tag="wk")
                nc.scalar.dma_start(out=t[:], in_=Wk[dc * 128:(dc + 1) * 128, :])
                wk_sb.append(t)
            for dc in range(DC):
                t = wkvp.tile([128, KVPC * DH], BF16, tag="wv")
                nc.scalar.dma_start(out=t[:], in_=Wv[dc * 128:(dc + 1) * 128, :])
                wv_sb.append(t)
            # wo rides the idle gpsimd queue: it is not needed until the
            # first out-projection (stage 1), and on sync it would delay
            # the xs streams.
            wo_sb = {}
            for j in range(4):
                for ob in range(4):
                    t = wop.tile([128, NBW], BF16, tag="wo")
                    nc.gpsimd.dma_start(
                        out=t[:],
                        in_=Wo[j * 128:(j + 1) * 128, ob * NBW:(ob + 1) * NBW])
                    wo_sb[(j, ob)] = t

            # ---- filler machinery ----------------------------------------
            # Thunks emitting PE-heavy work between attention ops so the
            # tensor engine never starves while ACT runs exp. hi = next
            # block's projections (must drain before that block's attention);
            # lo = previous block's out-projection (can carry over stages).
            # Queues hold (pe_cost_ns, fn). Emission is budget-paced: during
            # attention the ACT exp stream is the pacer, so filler is
            # released only up to the ACT-minus-attention-PE slack --
            # anything more would delay later scores and starve ACT;
            # anything less idles the PE and risks a HAM re-throttle.
            filler_hi = []
            filler_lo = []
            clock = {"act": 0.0, "pe": 0.0}

            # Keep a ~2.5us PE backlog on top of the ACT pacing: the HAM
            # clock gate only returns to full speed after a ~3.4us window of
            # CONTIGUOUS PE activity, so an exactly-paced schedule (PE ~80%
            # busy with frequent sub-us waits) that ever throttles stays at
            # half clock forever. A standing backlog keeps the PE densely
            # busy and recoverable.
            def emit_filler_budget(allow_lo=False):
                while filler_hi or (allow_lo and filler_lo):
                    q = filler_hi if filler_hi else filler_lo
                    cost, fn = q[0][0], q[0][1]
                    if clock["act"] + 2500.0 - clock["pe"] < cost:
                        break
                    q.pop(0)
                    clock["pe"] += cost
                    fn()

            # Alternate filler psum allocations between the two filler banks
            # so a unit's copy-out overlaps the next unit's matmuls instead
            # of write-after-read blocking on a single bank.
            fill_ctr = [0]

            def fill_ps_tile():
                fill_ctr[0] += 1
                if fill_ctr[0] % 2:
                    return msp.tile([128, NBW], F32, tag="ms", name="fps")
                return pjp.tile([128, NBW], F32, tag="pj", name="fps")

            def drain_hi():
                while filler_hi:
                    cost, fn, _key = filler_hi.pop(0)
                    clock["pe"] += cost
                    fn()

            def force_hi(b, level):
                # Pull queued projection work this attention point depends
                # on. Thunks are keyed (block, level); the hi queue is FIFO
                # and level-ordered within a block, so front-popping
                # suffices.
                while filler_hi:
                    key = filler_hi[0][2]
                    if key is None or key[0] != b or key[1] > level:
                        break
                    cost, fn, _key = filler_hi.pop(0)
                    clock["pe"] += cost
                    fn()

            xs_tiles = {}          # b -> list of 16 sbuf tiles

            def emit_xs_dma(b):
                ts = []
                for dc in range(DC):
                    t = xsp.tile([128, NBW], BF16, tag="xs", name="xs")
                    nc.sync.dma_start(
                        out=t[:],
                        in_=xT[dc * 128:(dc + 1) * 128,
                               b * NBW:(b + 1) * NBW])
                    ts.append(t)
                xs_tiles[b] = ts

            def proj_thunks(b):
                """Projection of q-block b: 6 bank-passes over resident xs,
                each pass split into 4-dc chunks (~850ns PE). Pass order
                [q0, k, v, q1, q2, q3] matches the gate order attention(b)
                pulls them in: pair g needs q-slab g; the k/v of block b are
                first needed at key-block 4b (the diagonal region)."""
                thunks = []

                def qk_chunk(g, c0, cell):
                    # g in 0..3 -> q slab g ; g == 4 -> k
                    def mk():
                        xs = xs_tiles[b]
                        if c0 == 0:
                            cell.append(fill_ps_tile())
                        ps = cell[0]
                        for dc in range(c0, c0 + 4):
                            stat = (wq_sb[dc][:, g * 128:(g + 1) * 128]
                                    if g < 4 else wk_sb[dc][:])
                            nc.tensor.matmul(ps[:], stat, xs[dc][:],
                                             start=(dc == 0),
                                             stop=(dc == DC - 1))
                        if c0 + 4 == DC:
                            dst = qt[g] if g < 4 else kt
                            nc.vector.tensor_copy(
                                dst[:, b * NBW:(b + 1) * NBW], ps[:])
                    return mk

                def v_chunk(c0, cell):
                    def mk():
                        xs = xs_tiles[b]
                        if c0 == 0:
                            cell.append(fill_ps_tile())
                        ps = cell[0]
                        for dc in range(c0, c0 + 4):
                            for i in range(4):
                                nc.tensor.matmul(
                                    ps[:, i * 128:(i + 1) * 128],
                                    xs[dc][:, i * 128:(i + 1) * 128],
                                    wv_sb[dc][:],
                                    start=(dc == 0 and i == 0),
                                    stop=(dc == DC - 1 and i == 3),
                                    skip_group_check=True)
                        if c0 + 4 == DC:
                            for i in range(4):
                                m = b * 4 + i
                                nc.vector.tensor_copy(
                                    va[m][:, 0:DH],
                                    ps[:, i * 128:i * 128 + 64])
                                nc.vector.tensor_copy(
                                    va[m][:, DH + 1:2 * DH + 1],
                                    ps[:, i * 128 + 64:i * 128 + 128])
                    return mk

                def qk_pass(g, level):
                    cell = []
                    return [(1010, qk_chunk(g, c0, cell), (b, level))
                            for c0 in range(0, DC, 4)]

                for g in range(4):
                    thunks += qk_pass(g, g)
                thunks += qk_pass(4, 4)
                cell = []
                thunks += [(1490, v_chunk(c0, cell), (b, 4))
                           for c0 in range(0, DC, 4)]
                return thunks

            def outproj_thunks(b):
                """Out-projection of q-block b (4 row tiles)."""
                thunks = []

                def ob_unit(nt, ob, osb):
                    def mk():
                        ops = fill_ps_tile()
                        for j in range(4):
                            nc.tensor.matmul(
                                ops[:],
                                ctxT[j][:, nt * 128:(nt + 1) * 128],
                                wo_sb[(j, ob)][:],
                                start=(j == 0), stop=(j == 3))
                        nc.vector.tensor_copy(
                            osb[0][:, ob * NBW:(ob + 1) * NBW], ops[:])
                    return mk

                def out_dma(nt, osb):
                    def mk():
                        nc.gpsimd.dma_start(
                            out=OUT[nt * 128:(nt + 1) * 128, :], in_=osb[0][:])
                    return mk

                for i in range(4):
                    nt = b * 4 + i
                    osb = []

                    def alloc(osb=osb):
                        osb.append(outp.tile([128, D], F32, tag="osb",
                                             name="osb"))
                    thunks.append((0, alloc, None))
                    for ob in range(4):
                        thunks.append((1010, ob_unit(nt, ob, osb), None))
                    thunks.append((0, out_dma(nt, osb), None))
                return thunks

            # ---- norm: ctx / softmax-denominator, into ctxT ---------------
            def emit_pair_norm(c_a, c_b, g, q0):
                # Critical prefix: 4 plain copies move the ctx rows and the
                # denominator rows (psum partition 64, from the ones column
                # of va) out of PSUM so the two c banks free immediately.
                # Everything after runs async: broadcast both denominators
                # into one psum bank (col groups 0/64), 1/x as exp(-ln x)
                # on ACT (same table set as the attention exp; DVE
                # reciprocal costs ~3.3us per call), then normalize. Head
                # B's rows reach partitions 64:128 via a sbuf-to-sbuf DMA
                # (engines cannot shift partitions).
                lrowA = smp.tile([65, NBW], F32R, tag="lrA", name="lrA")
                nc.vector.tensor_copy(lrowA[DH:DH + 1, :], c_a[DH:DH + 1, :])
                lrowB = smp.tile([65, NBW], F32R, tag="lrB", name="lrB")
                nc.vector.tensor_copy(lrowB[DH:DH + 1, :], c_b[DH:DH + 1, :])
                cuA = smp.tile([DH, NBW], F32, tag="cuA", name="cuA")
                nc.vector.tensor_copy(cuA[:], c_a[0:DH, :])
                cuB = smp.tile([DH, NBW], F32, tag="cuB", name="cuB")
                nc.vector.tensor_copy(cuB[:], c_b[0:DH, :])
                for lrow, cu, par in ((lrowA, cuA, 0), (lrowB, cuB, 1)):
                    rb_ps = msp.tile([DH, NBW], F32, tag="ms", name="rbps")
                    nc.tensor.matmul(rb_ps[:], ones_r[DH:DH + 1, 0:DH],
                                     lrow[DH:DH + 1, :], start=True, stop=True)
                    lg = smp.tile([DH, NBW], F32, tag=f"lg{par}", name="lg")
                    nc.scalar.activation(lg[:], rb_ps[:], LN)
                    rb = smp.tile([DH, NBW], F32, tag=f"rb{par}", name="rb")
                    nc.scalar.activation(rb[:], lg[:], EXP, scale=-1.0)
                    if par == 0:
                        nc.vector.tensor_mul(ctxT[g][0:DH, q0:q0 + NBW],
                                             cu[:], rb[:])
                    else:
                        tmp = smp.tile([DH, NBW], BF16, tag="ctmp",
                                       name="ctmp")
                        nc.vector.tensor_mul(tmp[:], cu[:], rb[:])
                        nc.sync.dma_start(
                            out=ctxT[g][DH:2 * DH, q0:q0 + NBW], in_=tmp[:])

            # ---- attention for one q-block, with filler interleave --------
            def attention(b):
                q0 = b * NBW
                n_kb = 4 * b + 4
                for g in range(4):
                    c_a = cpp.tile([DH + 1, NBW], F32, tag="c", name="ca")
                    c_b = cpp.tile([DH + 1, NBW], F32, tag="c", name="cb")
                    pend = None
                    for kb in range(n_kb):
                        m0 = kb * 128
                        diag = kb >= 4 * b
                        off = max(0, m0 - q0)
                        T = scp.tile([128, 2 * NBW], F32, tag="sc", name="T")
                        # scores: kv0 rows 0:64 / kv1 rows 64:128 run as
                        # concurrent PE row-groups. Head A only needs cols
                        # >= off (earlier cols belong to fully-masked
                        # queries); head B keeps full width so the exp span
                        # [off : 2*NBW] reads initialized psum only.
                        nc.tensor.matmul(T[:, off:NBW],
                                         kt[0:64, m0:m0 + 128],
                                         qt[g][0:64, q0 + off:q0 + NBW],
                                         start=True, stop=not diag,
                                         skip_group_check=True)
                        nc.tensor.matmul(T[:, NBW:2 * NBW],
                                         kt[64:128, m0:m0 + 128],
                                         qt[g][64:128, q0:q0 + NBW],
                                         start=True, stop=not diag,
                                         skip_group_check=True)
                        if diag:
                            nc.tensor.matmul(T[:, off:off + 128],
                                             ident[:], maskneg[:],
                                             start=False, stop=True,
                                             skip_group_check=True)
                            nc.tensor.matmul(T[:, NBW + off:NBW + off + 128],
                                             ident[:], maskneg[:],
                                             start=False, stop=True,
                                             skip_group_check=True)
                        p = psbp.tile([128, 2 * NBW], BF16, tag="p", name="p")
                        nc.scalar.activation(p[:, off:2 * NBW],
                                             T[:, off:2 * NBW], EXP,
                                             scale=float(scale))
                        w = NBW - off
                        clock["act"] += (2 * NBW - off + 172) / 1.2 + 120
                        clock["pe"] += 3 * (w / 2.4 + 45)
                        if diag:
                            clock["pe"] += 2 * (128 / 2.4 + 45)
                        if pend is not None:
                            pend()
                        emit_filler_budget(allow_lo=(b == NB - 1))

                        def _ctx(kb=kb, off=off, p=p, c_a=c_a, c_b=c_b):
                            nc.tensor.matmul(
                                c_a[:, off:NBW],
                                va[kb][:, 0:DH + 1],
                                p[:, off:NBW],
                                start=(kb == 0), stop=(kb == n_kb - 1),
                                skip_group_check=True)
                            nc.tensor.matmul(
                                c_b[:, off:NBW],
                                va[kb][:, DH + 1:2 * DH + 2],
                                p[:, NBW + off:2 * NBW],
                                start=(kb == 0), stop=(kb == n_kb - 1),
                                skip_group_check=True)
                        pend = _ctx
                    pend()
                    emit_pair_norm(c_a, c_b, g, q0)
                    clock["act"] += 2 * ((NBW + 172) / 1.2 + 120)
                    clock["pe"] += 2 * (NBW / 2.4 + 45)
                    emit_filler_budget(allow_lo=(b == NB - 1))

            # ---- main schedule -------------------------------------------
            emit_xs_dma(0)
            for _cost, th, _key in proj_thunks(0):
                th()
            emit_xs_dma(1)
            for b in range(NB):
                if b + 1 < NB:
                    filler_hi.extend(proj_thunks(b + 1))
                if b + 2 < NB:
                    filler_hi.append(
                        (0, lambda b=b: emit_xs_dma(b + 2), (b + 2, -1)))
                if b >= 1:
                    filler_lo.extend(outproj_thunks(b - 1))
                attention(b)
                drain_hi()
                # stage boundary: queues sync up during the pure-PE drain
                clock["act"] = clock["pe"] = max(clock["act"], clock["pe"])
            # Tail: flush leftover lo first (its ctxT inputs are ready), then
            # the last block's out-projection; bank alternation keeps the PE
            # dense so the HAM gate can recover full clock.
            for _cost, fn, _key in filler_lo:
                fn()
            for _cost, fn, _key in outproj_thunks(NB - 1):
                fn()

    nc.compile()
    return nc


def kernel(x, Wq, Wk, Wv, Wo, bo):
    x = np.asarray(x, dtype=np.float32)
    Wq = np.asarray(Wq, dtype=np.float32)
    Wk = np.asarray(Wk, dtype=np.float32)
    Wv = np.asarray(Wv, dtype=np.float32)
    Wo = np.asarray(Wo, dtype=np.float32)
    bo = np.asarray(bo, dtype=np.float32)

    if "nc" not in _CACHED:
        _CACHED["nc"] = _build()
    nc = _CACHED["nc"]

    bf = ml_dtypes.bfloat16
    in_maps = []
    for c in range(8):
        b, t = c // 4, c % 4
        xTc = np.ascontiguousarray(x[b].T).astype(bf)
        # q slab g holds [kv-head 2t head g | kv-head 2t+1 head g]
        qcols = []
        for g in range(4):
            for kvl in range(KVPC):
                h = (2 * t + kvl) * G + g
                qcols.append(Wq[:, h * DH:(h + 1) * DH])
        wq_c = np.ascontiguousarray(np.concatenate(qcols, axis=1)).astype(bf)
        wk_c = np.ascontiguousarray(Wk[:, t * 128:(t + 1) * 128]).astype(bf)
        wv_c = np.ascontiguousarray(Wv[:, t * 128:(t + 1) * 128]).astype(bf)
        # Wo rows must follow the ctxT head-pair layout: slab g holds
        # [head (kv 2t, g) | head (kv 2t+1, g)]
        wrows = []
        for g in range(4):
            for kvl in range(KVPC):
                h = (2 * t + kvl) * G + g
                wrows.append(Wo[h * DH:(h + 1) * DH, :])
        wo_c = np.ascontiguousarray(np.concatenate(wrows, axis=0)).astype(bf)
        in_maps.append({"xT": xTc, "Wq": wq_c, "Wk": wk_c, "Wv": wv_c,
                        "Wo": wo_c})

    trace = bool(int(os.environ.get("GQA_TRACE", "0")))
    kwargs = {}
    if trace:
        import tempfile
        td = os.environ.get("GQA_TRACE_DIR") or tempfile.mkdtemp(prefix="gqa_")
        kwargs = dict(trace=True, tmpdir=td)
    res = run_bass_kernel_spmd(nc, in_maps, list(range(8)), **kwargs)
    _CACHED["last_result"] = res

    out = np.empty((B, N, D), dtype=np.float32)
    for b in range(B):
        acc = res.results[4 * b]["out"].astype(np.float32)
        for t in range(1, 4):
            acc = acc + res.results[4 * b + t]["out"]
        out[b] = acc + bo[None, :]
    return out


# revision 49
# speedup vs baseline: 1.1049x; 1.0046x over previous
"""GQA forward (B=2,N=2048,D=2048,H=32,KV=8,DH=64, causal) on 8 trn2 cores.

Sharding: 2-way data parallel over batch x 4-way tensor parallel over heads
(each core: 8 q-heads = 2 kv-heads, keeping group structure). Row-parallel
out-proj; the all-reduce over the 4 TP shards (+ bias) happens on host at
gather time.

v2 design (vs baseline three serial phases):
  - all matmul operands bf16 (fp32 PSUM accumulation) -> FWL weight loads,
    half DMA/SBUF traffic, no fp32r narrow-moving penalty.
  - one fused pipeline: projection of q-block b+1 and out-projection of
    q-block b-1 are interleaved as PE filler between the attention matmuls
    of q-block b, so the tensor engine never idles long enough for the HAM
    clock gate to re-throttle to 1.2 GHz (the baseline spent 389us at half
    clock during attention).
  - scores for the 2 kv-heads of a head-pair run concurrently in PE row
    groups (K=64 contractions at base partitions 0 / 64).
  - causal mask applied by accumulating an identity-matmul of a -30000
    constant onto the diagonal score blocks (no DVE in the exp->ctx path).
  - exp batched: one ACT instruction per [128, 1024] PSUM span (both heads
    of a pair for one key block).
  - V projected directly in [tokens, dh] orientation with xs chunks as the
    stationary operand (no PE transposes).
"""
import os
import sys

import numpy as np

if "/opt/trn_rl_repo" not in sys.path:
    sys.path.insert(0, "/opt/trn_rl_repo")

import ml_dtypes

import concourse.bacc as bacc
import concourse.tile as tile
from concourse import mybir
from concourse.bass_utils import run_bass_kernel_spmd
from concourse.masks import make_identity

F32 = mybir.dt.float32
F32R = mybir.dt.float32r
BF16 = mybir.dt.bfloat16
EXP = mybir.ActivationFunctionType.Exp
LN = mybir.ActivationFunctionType.Ln

B, N, D = 2, 2048, 2048
H, KV, DH = 32, 8, 64
G = H // KV                      # 4 q-heads per kv head
HPC, KVPC = 8, 2                 # heads / kv-heads per core
DQ = HPC * DH                    # 512 per-core q projection width
NBW = 512                        # q-block width
NB = N // NBW                    # 4 q-blocks
DC = D // 128                    # 16 contraction chunks
NEG = -30000.0                   # causal mask additive constant

_CACHED = {}


def _build():
    nc = bacc.Bacc("TRN2", target_bir_lowering=False, debug=False,
                   num_devices=8)

    # Both Exp and Ln are used (softmax exp; 1/x as exp(-ln x)). The table
    # picker takes the first set containing each function, which would
    # alternate exp_and_others <-> natural_log at ~2.7us per switch. Trim
    # the cached table map (values only -- keys/order define act_func_set
    # ids and must stay) so the only set advertising Exp/Ln is the combined
    # one; it is then loaded exactly once.
    from concourse.hw_specs import get_activation_tables
    tabs = get_activation_tables(nc.m.arch)
    comb = tabs.get("natural_log_exp_and_others")
    if comb and EXP in comb and LN in comb:
        for name, fns in tabs.items():
            if name != "natural_log_exp_and_others":
                fns.discard(EXP)
                fns.discard(LN)

    xT = nc.dram_tensor("xT", [D, N], BF16, kind="ExternalInput")
    Wq = nc.dram_tensor("Wq", [D, DQ], BF16, kind="ExternalInput")
    Wk = nc.dram_tensor("Wk", [D, KVPC * DH], BF16, kind="ExternalInput")
    Wv = nc.dram_tensor("Wv", [D, KVPC * DH], BF16, kind="ExternalInput")
    Wo = nc.dram_tensor("Wo", [DQ, D], BF16, kind="ExternalInput")
    OUT = nc.dram_tensor("out", [N, D], F32, kind="ExternalOutput")

    scale = 1.0 / np.sqrt(DH)

    with tile.TileContext(nc) as tc:
        with (
            tc.tile_pool(name="persist", bufs=1) as pp,
            tc.tile_pool(name="wq", bufs=16) as wqp,
            tc.tile_pool(name="wkv", bufs=32) as wkvp,
            tc.tile_pool(name="wo", bufs=16) as wop,
            tc.tile_pool(name="xs", bufs=16) as xsp,
            tc.tile_pool(name="psb", bufs=4) as psbp,
            tc.tile_pool(name="outs", bufs=3) as outp,
            tc.tile_pool(name="small", bufs=2) as smp,
            tc.tile_pool(name="sc_ps", bufs=2, space="PSUM") as scp,
            tc.tile_pool(name="c_ps", bufs=2, space="PSUM") as cpp,
            tc.tile_pool(name="pj_ps", bufs=1, space="PSUM") as pjp,
            tc.tile_pool(name="ms_ps", bufs=1, space="PSUM") as msp,
        ):
            # ---- persistent sbuf state ----
            identf = pp.tile([128, 128], F32, tag="identf")
            make_identity(nc, identf[:])
            ident = pp.tile([128, 128], BF16, tag="ident")
            nc.vector.tensor_copy(ident[:], identf[:])

            # additive causal mask for a 128x128 diagonal block:
            # mask[k, j] = 0 if j >= k else NEG (local query j, local key k)
            mknf = pp.tile([128, 128], F32, tag="mknf")
            nc.gpsimd.memset(mknf[:], 0.0)
            nc.gpsimd.affine_select(
                out=mknf[:], in_=mknf[:],
                compare_op=mybir.AluOpType.is_ge,
                fill=NEG, base=0,
                pattern=[[1, 128]],
                channel_multiplier=-1,
            )
            maskneg = pp.tile([128, 128], BF16, tag="maskneg")
            nc.vector.tensor_copy(maskneg[:], mknf[:])

            onesf = pp.tile([128, 64], F32, tag="onesf")
            nc.vector.memset(onesf[:], 1.0)
            ones_b = pp.tile([128, 64], BF16, tag="onesb")
            nc.vector.tensor_copy(ones_b[:], onesf[:])
            ones_r = pp.tile([128, 64], F32R, tag="onesr")
            nc.vector.tensor_copy(ones_r[:], onesf[:])

            qt = [pp.tile([128, N], BF16, tag=f"qt{g}", name=f"qt{g}")
                  for g in range(4)]
            kt = pp.tile([128, N], BF16, tag="kt")
            # va[m]: [ A_dh(0:64) | onesA(64) | B_dh(65:129) | onesB(129) ]
            va = [pp.tile([128, 2 * (DH + 1)], BF16, tag=f"va{m}",
                          name=f"va{m}") for m in range(N // 128)]
            for m in range(N // 128):
                nc.vector.tensor_copy(va[m][:, DH:DH + 1], ones_b[:, 0:1])
                nc.vector.tensor_copy(va[m][:, 2 * DH + 1:2 * DH + 2],
                                      ones_b[:, 0:1])
            ctxT = [pp.tile([128, N], BF16, tag=f"ct{g}", name=f"ct{g}")
                    for g in range(4)]

            # ---- weights ----
            wq_sb, wk_sb, wv_sb = [], [], []
            for dc in range(DC):
                t = wqp.tile([128, DQ], BF16, tag="w")
                eng = nc.scalar if dc % 2 == 0 else nc.gpsimd
                eng.dma_start(out=t[:], in_=Wq[dc * 128:(dc + 1) * 128, :])
                wq_sb.append(t)
            for dc in range(DC):
                t = wkvp.tile([128, KVPC * DH], BF16, tag="wk")
                nc.scalar.dma_start(out=t[:], in_=Wk[dc * 128:(dc + 1) * 128, :])
                wk_sb.append(t)
            for dc in range(DC):
                t = wkvp.tile([128, KVPC * DH], BF16, tag="wv")
                nc.scalar.dma_start(out=t[:], in_=Wv[dc * 128:(dc + 1) * 128, :])
                wv_sb.append(t)
            # wo rides the idle gpsimd queue: it is not needed until the
            # first out-projection (stage 1), and on sync it would delay
            # the xs streams.
            wo_sb = {}
            for j in range(4):
                for ob in range(4):
                    t = wop.tile([128, NBW], BF16, tag="wo")
                    nc.gpsimd.dma_start(
                        out=t[:],
                        in_=Wo[j * 128:(j + 1) * 128, ob * NBW:(ob + 1) * NBW])
                    wo_sb[(j, ob)] = t

            # ---- filler machinery ----------------------------------------
            # Thunks emitting PE-heavy work between attention ops so the
            # tensor engine never starves while ACT runs exp. hi = next
            # block's projections (must drain before that block's attention);
            # lo = previous block's out-projection (can carry over stages).
            # Queues hold (pe_cost_ns, fn). Emission is budget-paced: during
            # attention the ACT exp stream is the pacer, so filler is
            # released only up to the ACT-minus-attention-PE slack --
            # anything more would delay later scores and starve ACT;
            # anything less idles the PE and risks a HAM re-throttle.
            filler_hi = []
            filler_lo = []
            clock = {"act": 0.0, "pe": 0.0}

            # Keep a ~2.5us PE backlog on top of the ACT pacing: the HAM
            # clock gate only returns to full speed after a ~3.4us window of
            # CONTIGUOUS PE activity, so an exactly-paced schedule (PE ~80%
            # busy with frequent sub-us waits) that ever throttles stays at
            # half clock forever. A standing backlog keeps the PE densely
            # busy and recoverable.
            def emit_filler_budget(allow_lo=False):
                while filler_hi or (allow_lo and filler_lo):
                    q = filler_hi if filler_hi else filler_lo
                    cost, fn = q[0][0], q[0][1]
                    if clock["act"] + 4000.0 - clock["pe"] < cost:
                        break
                    q.pop(0)
                    clock["pe"] += cost
                    fn()

            # Alternate filler psum allocations between the two filler banks
            # so a unit's copy-out overlaps the next unit's matmuls instead
            # of write-after-read blocking on a single bank.
            fill_ctr = [0]

            def fill_ps_tile():
                fill_ctr[0] += 1
                if fill_ctr[0] % 2:
                    return msp.tile([128, NBW], F32, tag="ms", name="fps")
                return pjp.tile([128, NBW], F32, tag="pj", name="fps")

            def drain_hi():
                while filler_hi:
                    cost, fn, _key = filler_hi.pop(0)
                    clock["pe"] += cost
                    fn()

            def force_hi(b, level):
                # Pull queued projection work this attention point depends
                # on. Thunks are keyed (block, level); the hi queue is FIFO
                # and level-ordered within a block, so front-popping
                # suffices.
                while filler_hi:
                    key = filler_hi[0][2]
                    if key is None or key[0] != b or key[1] > level:
                        break
                    cost, fn, _key = filler_hi.pop(0)
                    clock["pe"] += cost
                    fn()

            xs_tiles = {}          # b -> list of 16 sbuf tiles

            def emit_xs_dma(b):
                ts = []
                for dc in range(DC):
                    t = xsp.tile([128, NBW], BF16, tag="xs", name="xs")
                    nc.sync.dma_start(
                        out=t[:],
                        in_=xT[dc * 128:(dc + 1) * 128,
                               b * NBW:(b + 1) * NBW])
                    ts.append(t)
                xs_tiles[b] = ts

            def proj_thunks(b):
                """Projection of q-block b: 6 bank-passes over resident xs,
                each pass split into 4-dc chunks (~850ns PE). Pass order
                [q0, k, v, q1, q2, q3] matches the gate order attention(b)
                pulls them in: pair g needs q-slab g; the k/v of block b are
                first needed at key-block 4b (the diagonal region)."""
                thunks = []

                def qk_chunk(g, c0, cell):
                    # g in 0..3 -> q slab g ; g == 4 -> k
                    def mk():
                        xs = xs_tiles[b]
                        if c0 == 0:
                            cell.append(fill_ps_tile())
                        ps = cell[0]
                        for dc in range(c0, c0 + 4):
                            stat = (wq_sb[dc][:, g * 128:(g + 1) * 128]
                                    if g < 4 else wk_sb[dc][:])
                            nc.tensor.matmul(ps[:], stat, xs[dc][:],
                                             start=(dc == 0),
                                             stop=(dc == DC - 1))
                        if c0 + 4 == DC:
                            dst = qt[g] if g < 4 else kt
                            nc.vector.tensor_copy(
                                dst[:, b * NBW:(b + 1) * NBW], ps[:])
                    return mk

                def v_chunk(c0, cell):
                    def mk():
                        xs = xs_tiles[b]
                        if c0 == 0:
                            cell.append(fill_ps_tile())
                        ps = cell[0]
                        for dc in range(c0, c0 + 4):
                            for i in range(4):
                                nc.tensor.matmul(
                                    ps[:, i * 128:(i + 1) * 128],
                                    xs[dc][:, i * 128:(i + 1) * 128],
                                    wv_sb[dc][:],
                                    start=(dc == 0 and i == 0),
                                    stop=(dc == DC - 1 and i == 3),
                                    skip_group_check=True)
                        if c0 + 4 == DC:
                            for i in range(4):
                                m = b * 4 + i
                                nc.vector.tensor_copy(
                                    va[m][:, 0:DH],
                                    ps[:, i * 128:i * 128 + 64])
                                nc.vector.tensor_copy(
                                    va[m][:, DH + 1:2 * DH + 1],
                                    ps[:, i * 128 + 64:i * 128 + 128])
                    return mk

                def qk_pass(g, level):
                    cell = []
                    return [(1010, qk_chunk(g, c0, cell), (b, level))
                            for c0 in range(0, DC, 4)]

                for g in range(4):
                    thunks += qk_pass(g, g)
                thunks += qk_pass(4, 4)
                cell = []
                thunks += [(1490, v_chunk(c0, cell), (b, 4))
                           for c0 in range(0, DC, 4)]
                return thunks

            def outproj_thunks(b):
                """Out-projection of q-block b (4 row tiles)."""
                thunks = []

                def ob_unit(nt, ob, osb):
                    def mk():
                        ops = fill_ps_tile()
                        for j in range(4):
                            nc.tensor.matmul(
                                ops[:],
                                ctxT[j][:, nt * 128:(nt + 1) * 128],
                                wo_sb[(j, ob)][:],
                                start=(j == 0), stop=(j == 3))
                        nc.vector.tensor_copy(
                            osb[0][:, ob * NBW:(ob + 1) * NBW], ops[:])
                    return mk

                def out_dma(nt, osb):
                    def mk():
                        # alternate queues: 16x1MB on one queue serializes
                        # the tail
                        eng = nc.gpsimd if nt % 2 == 0 else nc.sync
                        eng.dma_start(
                            out=OUT[nt * 128:(nt + 1) * 128, :], in_=osb[0][:])
                    return mk

                for i in range(4):
                    nt = b * 4 + i
                    osb = []

                    def alloc(osb=osb):
                        osb.append(outp.tile([128, D], F32, tag="osb",
                                             name="osb"))
                    thunks.append((0, alloc, None))
                    for ob in range(4):
                        thunks.append((1010, ob_unit(nt, ob, osb), None))
                    thunks.append((0, out_dma(nt, osb), None))
                return thunks

            # ---- norm: ctx / softmax-denominator, into ctxT ---------------
            def emit_pair_norm(c_a, c_b, g, q0):
                # Critical prefix: 4 plain copies move the ctx rows and the
                # denominator rows (psum partition 64, from the ones column
                # of va) out of PSUM so the two c banks free immediately.
                # Everything after runs async: broadcast both denominators
                # into one psum bank (col groups 0/64), 1/x as exp(-ln x)
                # on ACT (same table set as the attention exp; DVE
                # reciprocal costs ~3.3us per call), then normalize. Head
                # B's rows reach partitions 64:128 via a sbuf-to-sbuf DMA
                # (engines cannot shift partitions).
                lrowA = smp.tile([65, NBW], F32R, tag="lrA", name="lrA")
                nc.vector.tensor_copy(lrowA[DH:DH + 1, :], c_a[DH:DH + 1, :])
                lrowB = smp.tile([65, NBW], F32R, tag="lrB", name="lrB")
                nc.vector.tensor_copy(lrowB[DH:DH + 1, :], c_b[DH:DH + 1, :])
                cuA = smp.tile([DH, NBW], F32, tag="cuA", name="cuA")
                nc.vector.tensor_copy(cuA[:], c_a[0:DH, :])
                cuB = smp.tile([DH, NBW], F32, tag="cuB", name="cuB")
                nc.vector.tensor_copy(cuB[:], c_b[0:DH, :])
                for lrow, cu, par in ((lrowA, cuA, 0), (lrowB, cuB, 1)):
                    rb_ps = msp.tile([DH, NBW], F32, tag="ms", name="rbps")
                    nc.tensor.matmul(rb_ps[:], ones_r[DH:DH + 1, 0:DH],
                                     lrow[DH:DH + 1, :], start=True, stop=True)
                    lg = smp.tile([DH, NBW], F32, tag=f"lg{par}", name="lg")
                    nc.scalar.activation(lg[:], rb_ps[:], LN)
                    rb = smp.tile([DH, NBW], F32, tag=f"rb{par}", name="rb")
                    nc.scalar.activation(rb[:], lg[:], EXP, scale=-1.0)
                    if par == 0:
                        nc.vector.tensor_mul(ctxT[g][0:DH, q0:q0 + NBW],
                                             cu[:], rb[:])
                    else:
                        tmp = smp.tile([DH, NBW], BF16, tag="ctmp",
                                       name="ctmp")
                        nc.vector.tensor_mul(tmp[:], cu[:], rb[:])
                        nc.sync.dma_start(
                            out=ctxT[g][DH:2 * DH, q0:q0 + NBW], in_=tmp[:])

            # ---- attention for one q-block, with filler interleave --------
            def attention(b):
                q0 = b * NBW
                n_kb = 4 * b + 4
                for g in range(4):
                    c_a = cpp.tile([DH + 1, NBW], F32, tag="c", name="ca")
                    c_b = cpp.tile([DH + 1, NBW], F32, tag="c", name="cb")
                    pend = None
                    for kb in range(n_kb):
                        m0 = kb * 128
                        diag = kb >= 4 * b
                        off = max(0, m0 - q0)
                        T = scp.tile([128, 2 * NBW], F32, tag="sc", name="T")
                        # scores: kv0 rows 0:64 / kv1 rows 64:128 run as
                        # concurrent PE row-groups. Head A only needs cols
                        # >= off (earlier cols belong to fully-masked
                        # queries); head B keeps full width so the exp span
                        # [off : 2*NBW] reads initialized psum only.
                        nc.tensor.matmul(T[:, off:NBW],
                                         kt[0:64, m0:m0 + 128],
                                         qt[g][0:64, q0 + off:q0 + NBW],
                                         start=True, stop=not diag,
                                         skip_group_check=True)
                        nc.tensor.matmul(T[:, NBW:2 * NBW],
                                         kt[64:128, m0:m0 + 128],
                                         qt[g][64:128, q0:q0 + NBW],
                                         start=True, stop=not diag,
                                         skip_group_check=True)
                        if diag:
                            nc.tensor.matmul(T[:, off:off + 128],
                                             ident[:], maskneg[:],
                                             start=False, stop=True,
                                             skip_group_check=True)
                            nc.tensor.matmul(T[:, NBW + off:NBW + off + 128],
                                             ident[:], maskneg[:],
                                             start=False, stop=True,
                                             skip_group_check=True)
                        p = psbp.tile([128, 2 * NBW], BF16, tag="p", name="p")
                        nc.scalar.activation(p[:, off:2 * NBW],
                                             T[:, off:2 * NBW], EXP,
                                             scale=float(scale))
                        w = NBW - off
                        clock["act"] += (2 * NBW - off + 172) / 1.2 + 120
                        clock["pe"] += 3 * (w / 2.4 + 45)
                        if diag:
                            clock["pe"] += 2 * (128 / 2.4 + 45)
                        if pend is not None:
                            pend()
                        emit_filler_budget(allow_lo=(b == NB - 1))

                        def _ctx(kb=kb, off=off, p=p, c_a=c_a, c_b=c_b):
                            nc.tensor.matmul(
                                c_a[:, off:NBW],
                                va[kb][:, 0:DH + 1],
                                p[:, off:NBW],
                                start=(kb == 0), stop=(kb == n_kb - 1),
                                skip_group_check=True)
                            nc.tensor.matmul(
                                c_b[:, off:NBW],
                                va[kb][:, DH + 1:2 * DH + 2],
                                p[:, NBW + off:2 * NBW],
                                start=(kb == 0), stop=(kb == n_kb - 1),
                                skip_group_check=True)
                        pend = _ctx
                    pend()
                    emit_pair_norm(c_a, c_b, g, q0)
                    clock["act"] += 2 * ((NBW + 172) / 1.2 + 120)
                    clock["pe"] += 2 * (NBW / 2.4 + 45)
                    emit_filler_budget(allow_lo=(b == NB - 1))

            # ---- main schedule -------------------------------------------
            emit_xs_dma(0)
            for _cost, th, _key in proj_thunks(0):
                th()
            emit_xs_dma(1)
            for b in range(NB):
                if b + 1 < NB:
                    filler_hi.extend(proj_thunks(b + 1))
                if b + 2 < NB:
                    filler_hi.append(
                        (0, lambda b=b: emit_xs_dma(b + 2), (b + 2, -1)))
                if b >= 1:
                    filler_lo.extend(outproj_thunks(b - 1))
                attention(b)
                drain_hi()
                # stage boundary: queues sync up during the pure-PE drain
                clock["act"] = clock["pe"] = max(clock["act"], clock["pe"])
            # Tail: flush leftover lo first (its ctxT inputs are ready), then
            # the last block's out-projection; bank alternation keeps the PE
            # dense so the HAM gate can recover full clock.
            for _cost, fn, _key in filler_lo:
                fn()
            for _cost, fn, _key in outproj_thunks(NB - 1):
                fn()

    nc.compile()
    return nc


def kernel(x, Wq, Wk, Wv, Wo, bo):
    x = np.asarray(x, dtype=np.float32)
    Wq = np.asarray(Wq, dtype=np.float32)
    Wk = np.asarray(Wk, dtype=np.float32)
    Wv = np.asarray(Wv, dtype=np.float32)
    Wo = np.asarray(Wo, dtype=np.float32)
    bo = np.asarray(bo, dtype=np.float32)

    if "nc" not in _CACHED:
        _CACHED["nc"] = _build()
    nc = _CACHED["nc"]

    bf = ml_dtypes.bfloat16
    in_maps = []
    for c in range(8):
        b, t = c // 4, c % 4
        xTc = np.ascontiguousarray(x[b].T).astype(bf)
        # q slab g holds [kv-head 2t head g | kv-head 2t+1 head g]
        qcols = []
        for g in range(4):
            for kvl in range(KVPC):
                h = (2 * t + kvl) * G + g
                qcols.append(Wq[:, h * DH:(h + 1) * DH])
        wq_c = np.ascontiguousarray(np.concatenate(qcols, axis=1)).astype(bf)
        wk_c = np.ascontiguousarray(Wk[:, t * 128:(t + 1) * 128]).astype(bf)
        wv_c = np.ascontiguousarray(Wv[:, t * 128:(t + 1) * 128]).astype(bf)
        # Wo rows must follow the ctxT head-pair layout: slab g holds
        # [head (kv 2t, g) | head (kv 2t+1, g)]
        wrows = []
        for g in range(4):
            for kvl in range(KVPC):
                h = (2 * t + kvl) * G + g
                wrows.append(Wo[h * DH:(h + 1) * DH, :])
        wo_c = np.ascontiguousarray(np.concatenate(wrows, axis=0)).astype(bf)
        in_maps.append({"xT": xTc, "Wq": wq_c, "Wk": wk_c, "Wv": wv_c,
                        "Wo": wo_c})

    trace = bool(int(os.environ.get("GQA_TRACE", "0")))
    kwargs = {}
    if trace:
        import tempfile
        td = os.environ.get("GQA_TRACE_DIR") or tempfile.mkdtemp(prefix="gqa_")
        kwargs = dict(trace=True, tmpdir=td)
    res = run_bass_kernel_spmd(nc, in_maps, list(range(8)), **kwargs)
    _CACHED["last_result"] = res

    out = np.empty((B, N, D), dtype=np.float32)
    for b in range(B):
        acc = res.results[4 * b]["out"].astype(np.float32)
        for t in range(1, 4):
            acc = acc + res.results[4 * b + t]["out"]
        out[b] = acc + bo[None, :]
    return out


# revision 51
# speedup vs baseline: 1.1167x; 1.0106x over previous
"""GQA forward (B=2,N=2048,D=2048,H=32,KV=8,DH=64, causal) on 8 trn2 cores.

Sharding: 2-way data parallel over batch x 4-way tensor parallel over heads
(each core: 8 q-heads = 2 kv-heads, keeping group structure). Row-parallel
out-proj; the all-reduce over the 4 TP shards (+ bias) happens on host at
gather time.

v2 design (vs baseline three serial phases):
  - all matmul operands bf16 (fp32 PSUM accumulation) -> FWL weight loads,
    half DMA/SBUF traffic, no fp32r narrow-moving penalty.
  - one fused pipeline: projection of q-block b+1 and out-projection of
    q-block b-1 are interleaved as PE filler between the attention matmuls
    of q-block b, so the tensor engine never idles long enough for the HAM
    clock gate to re-throttle to 1.2 GHz (the baseline spent 389us at half
    clock during attention).
  - scores for the 2 kv-heads of a head-pair run concurrently in PE row
    groups (K=64 contractions at base partitions 0 / 64).
  - causal mask applied by accumulating an identity-matmul of a -30000
    constant onto the diagonal score blocks (no DVE in the exp->ctx path).
  - exp batched: one ACT instruction per [128, 1024] PSUM span (both heads
    of a pair for one key block).
  - V projected directly in [tokens, dh] orientation with xs chunks as the
    stationary operand (no PE transposes).
"""
import os
import sys

import numpy as np

if "/opt/trn_rl_repo" not in sys.path:
    sys.path.insert(0, "/opt/trn_rl_repo")

import ml_dtypes

import concourse.bacc as bacc
import concourse.tile as tile
from concourse import mybir
from concourse.bass_utils import run_bass_kernel_spmd
from concourse.masks import make_identity

F32 = mybir.dt.float32
F32R = mybir.dt.float32r
BF16 = mybir.dt.bfloat16
EXP = mybir.ActivationFunctionType.Exp
LN = mybir.ActivationFunctionType.Ln

B, N, D = 2, 2048, 2048
H, KV, DH = 32, 8, 64
G = H // KV                      # 4 q-heads per kv head
HPC, KVPC = 8, 2                 # heads / kv-heads per core
DQ = HPC * DH                    # 512 per-core q projection width
NBW = 512                        # q-block width
NB = N // NBW                    # 4 q-blocks
DC = D // 128                    # 16 contraction chunks
NEG = -30000.0                   # causal mask additive constant

_CACHED = {}


def _build():
    nc = bacc.Bacc("TRN2", target_bir_lowering=False, debug=False,
                   num_devices=8)

    # Both Exp and Ln are used (softmax exp; 1/x as exp(-ln x)). The table
    # picker takes the first set containing each function, which would
    # alternate exp_and_others <-> natural_log at ~2.7us per switch. Trim
    # the cached table map (values only -- keys/order define act_func_set
    # ids and must stay) so the only set advertising Exp/Ln is the combined
    # one; it is then loaded exactly once.
    from concourse.hw_specs import get_activation_tables
    tabs = get_activation_tables(nc.m.arch)
    comb = tabs.get("natural_log_exp_and_others")
    if comb and EXP in comb and LN in comb:
        for name, fns in tabs.items():
            if name != "natural_log_exp_and_others":
                fns.discard(EXP)
                fns.discard(LN)

    xT = nc.dram_tensor("xT", [D, N], BF16, kind="ExternalInput")
    Wq = nc.dram_tensor("Wq", [D, DQ], BF16, kind="ExternalInput")
    Wk = nc.dram_tensor("Wk", [D, KVPC * DH], BF16, kind="ExternalInput")
    Wv = nc.dram_tensor("Wv", [D, KVPC * DH], BF16, kind="ExternalInput")
    Wo = nc.dram_tensor("Wo", [DQ, D], BF16, kind="ExternalInput")
    OUT = nc.dram_tensor("out", [N, D], F32, kind="ExternalOutput")

    scale = 1.0 / np.sqrt(DH)

    with tile.TileContext(nc) as tc:
        with (
            tc.tile_pool(name="persist", bufs=1) as pp,
            tc.tile_pool(name="wq", bufs=16) as wqp,
            tc.tile_pool(name="wkv", bufs=32) as wkvp,
            tc.tile_pool(name="wo", bufs=16) as wop,
            tc.tile_pool(name="xs", bufs=16) as xsp,
            tc.tile_pool(name="psb", bufs=4) as psbp,
            tc.tile_pool(name="outs", bufs=3) as outp,
            tc.tile_pool(name="small", bufs=2) as smp,
            tc.tile_pool(name="sc_ps", bufs=2, space="PSUM") as scp,
            tc.tile_pool(name="c_ps", bufs=2, space="PSUM") as cpp,
            tc.tile_pool(name="pj_ps", bufs=1, space="PSUM") as pjp,
            tc.tile_pool(name="ms_ps", bufs=1, space="PSUM") as msp,
        ):
            # ---- weights first: the DMAs must hit their queues before the
            # gpsimd/DVE constant construction below, or the first
            # projection pass starts several us late ----
            # ---- weights ----
            wq_sb, wk_sb, wv_sb = [], [], []
            for dc in range(DC):
                t = wqp.tile([128, DQ], BF16, tag="w")
                eng = nc.scalar if dc % 2 == 0 else nc.gpsimd
                eng.dma_start(out=t[:], in_=Wq[dc * 128:(dc + 1) * 128, :])
                wq_sb.append(t)
            for dc in range(DC):
                t = wkvp.tile([128, KVPC * DH], BF16, tag="wk")
                nc.scalar.dma_start(out=t[:], in_=Wk[dc * 128:(dc + 1) * 128, :])
                wk_sb.append(t)
            for dc in range(DC):
                t = wkvp.tile([128, KVPC * DH], BF16, tag="wv")
                nc.scalar.dma_start(out=t[:], in_=Wv[dc * 128:(dc + 1) * 128, :])
                wv_sb.append(t)
            # wo rides the idle gpsimd queue: it is not needed until the
            # first out-projection (stage 1), and on sync it would delay
            # the xs streams.
            wo_sb = {}
            for j in range(4):
                for ob in range(4):
                    t = wop.tile([128, NBW], BF16, tag="wo")
                    nc.gpsimd.dma_start(
                        out=t[:],
                        in_=Wo[j * 128:(j + 1) * 128, ob * NBW:(ob + 1) * NBW])
                    wo_sb[(j, ob)] = t

            # ---- persistent sbuf state ----
            identf = pp.tile([128, 128], F32, tag="identf")
            make_identity(nc, identf[:])
            ident = pp.tile([128, 128], BF16, tag="ident")
            nc.vector.tensor_copy(ident[:], identf[:])

            # additive causal mask for a 128x128 diagonal block:
            # mask[k, j] = 0 if j >= k else NEG (local query j, local key k)
            mknf = pp.tile([128, 128], F32, tag="mknf")
            nc.gpsimd.memset(mknf[:], 0.0)
            nc.gpsimd.affine_select(
                out=mknf[:], in_=mknf[:],
                compare_op=mybir.AluOpType.is_ge,
                fill=NEG, base=0,
                pattern=[[1, 128]],
                channel_multiplier=-1,
            )
            maskneg = pp.tile([128, 128], BF16, tag="maskneg")
            nc.vector.tensor_copy(maskneg[:], mknf[:])

            onesf = pp.tile([128, 64], F32, tag="onesf")
            nc.vector.memset(onesf[:], 1.0)
            ones_b = pp.tile([128, 64], BF16, tag="onesb")
            nc.vector.tensor_copy(ones_b[:], onesf[:])
            ones_r = pp.tile([128, 64], F32R, tag="onesr")
            nc.vector.tensor_copy(ones_r[:], onesf[:])

            qt = [pp.tile([128, N], BF16, tag=f"qt{g}", name=f"qt{g}")
                  for g in range(4)]
            kt = pp.tile([128, N], BF16, tag="kt")
            # va[m]: [ A_dh(0:64) | onesA(64) | B_dh(65:129) | onesB(129) ]
            va = [pp.tile([128, 2 * (DH + 1)], BF16, tag=f"va{m}",
                          name=f"va{m}") for m in range(N // 128)]
            for m in range(N // 128):
                nc.vector.tensor_copy(va[m][:, DH:DH + 1], ones_b[:, 0:1])
                nc.vector.tensor_copy(va[m][:, 2 * DH + 1:2 * DH + 2],
                                      ones_b[:, 0:1])
            ctxT = [pp.tile([128, N], BF16, tag=f"ct{g}", name=f"ct{g}")
                    for g in range(4)]

            # ---- filler machinery ----------------------------------------
            # Thunks emitting PE-heavy work between attention ops so the
            # tensor engine never starves while ACT runs exp. hi = next
            # block's projections (must drain before that block's attention);
            # lo = previous block's out-projection (can carry over stages).
            # Queues hold (pe_cost_ns, fn). Emission is budget-paced: during
            # attention the ACT exp stream is the pacer, so filler is
            # released only up to the ACT-minus-attention-PE slack --
            # anything more would delay later scores and starve ACT;
            # anything less idles the PE and risks a HAM re-throttle.
            filler_hi = []
            filler_lo = []
            clock = {"act": 0.0, "pe": 0.0}

            # Keep a ~2.5us PE backlog on top of the ACT pacing: the HAM
            # clock gate only returns to full speed after a ~3.4us window of
            # CONTIGUOUS PE activity, so an exactly-paced schedule (PE ~80%
            # busy with frequent sub-us waits) that ever throttles stays at
            # half clock forever. A standing backlog keeps the PE densely
            # busy and recoverable.
            def emit_filler_budget(allow_lo=False):
                while filler_hi or (allow_lo and filler_lo):
                    q = filler_hi if filler_hi else filler_lo
                    cost, fn = q[0][0], q[0][1]
                    if clock["act"] + 4000.0 - clock["pe"] < cost:
                        break
                    q.pop(0)
                    clock["pe"] += cost
                    fn()

            # Alternate filler psum allocations between the two filler banks
            # so a unit's copy-out overlaps the next unit's matmuls instead
            # of write-after-read blocking on a single bank.
            fill_ctr = [0]

            def fill_ps_tile():
                fill_ctr[0] += 1
                if fill_ctr[0] % 2:
                    return msp.tile([128, NBW], F32, tag="ms", name="fps")
                return pjp.tile([128, NBW], F32, tag="pj", name="fps")

            def drain_hi():
                while filler_hi:
                    cost, fn, _key = filler_hi.pop(0)
                    clock["pe"] += cost
                    fn()

            def force_hi(b, level):
                # Pull queued projection work this attention point depends
                # on. Thunks are keyed (block, level); the hi queue is FIFO
                # and level-ordered within a block, so front-popping
                # suffices.
                while filler_hi:
                    key = filler_hi[0][2]
                    if key is None or key[0] != b or key[1] > level:
                        break
                    cost, fn, _key = filler_hi.pop(0)
                    clock["pe"] += cost
                    fn()

            xs_tiles = {}          # b -> list of 16 sbuf tiles

            def emit_xs_dma(b):
                ts = []
                for dc in range(DC):
                    t = xsp.tile([128, NBW], BF16, tag="xs", name="xs")
                    nc.sync.dma_start(
                        out=t[:],
                        in_=xT[dc * 128:(dc + 1) * 128,
                               b * NBW:(b + 1) * NBW])
                    ts.append(t)
                xs_tiles[b] = ts

            def proj_thunks(b):
                """Projection of q-block b: 6 bank-passes over resident xs,
                each pass split into 4-dc chunks (~850ns PE). Pass order
                [q0, k, v, q1, q2, q3] matches the gate order attention(b)
                pulls them in: pair g needs q-slab g; the k/v of block b are
                first needed at key-block 4b (the diagonal region)."""
                thunks = []

                def qk_chunk(g, c0, cell):
                    # g in 0..3 -> q slab g ; g == 4 -> k
                    def mk():
                        xs = xs_tiles[b]
                        if c0 == 0:
                            cell.append(fill_ps_tile())
                        ps = cell[0]
                        for dc in range(c0, c0 + 4):
                            stat = (wq_sb[dc][:, g * 128:(g + 1) * 128]
                                    if g < 4 else wk_sb[dc][:])
                            nc.tensor.matmul(ps[:], stat, xs[dc][:],
                                             start=(dc == 0),
                                             stop=(dc == DC - 1))
                        if c0 + 4 == DC:
                            dst = qt[g] if g < 4 else kt
                            nc.vector.tensor_copy(
                                dst[:, b * NBW:(b + 1) * NBW], ps[:])
                    return mk

                def v_chunk(c0, cell):
                    def mk():
                        xs = xs_tiles[b]
                        if c0 == 0:
                            cell.append(fill_ps_tile())
                        ps = cell[0]
                        for dc in range(c0, c0 + 4):
                            for i in range(4):
                                nc.tensor.matmul(
                                    ps[:, i * 128:(i + 1) * 128],
                                    xs[dc][:, i * 128:(i + 1) * 128],
                                    wv_sb[dc][:],
                                    start=(dc == 0 and i == 0),
                                    stop=(dc == DC - 1 and i == 3),
                                    skip_group_check=True)
                        if c0 + 4 == DC:
                            for i in range(4):
                                m = b * 4 + i
                                nc.vector.tensor_copy(
                                    va[m][:, 0:DH],
                                    ps[:, i * 128:i * 128 + 64])
                                nc.vector.tensor_copy(
                                    va[m][:, DH + 1:2 * DH + 1],
                                    ps[:, i * 128 + 64:i * 128 + 128])
                    return mk

                def qk_pass(g, level):
                    cell = []
                    return [(1010, qk_chunk(g, c0, cell), (b, level))
                            for c0 in range(0, DC, 4)]

                for g in range(4):
                    thunks += qk_pass(g, g)
                thunks += qk_pass(4, 4)
                cell = []
                thunks += [(1490, v_chunk(c0, cell), (b, 4))
                           for c0 in range(0, DC, 4)]
                return thunks

            def outproj_thunks(b):
                """Out-projection of q-block b (4 row tiles)."""
                thunks = []

                def ob_unit(nt, ob, osb):
                    def mk():
                        ops = fill_ps_tile()
                        for j in range(4):
                            nc.tensor.matmul(
                                ops[:],
                                ctxT[j][:, nt * 128:(nt + 1) * 128],
                                wo_sb[(j, ob)][:],
                                start=(j == 0), stop=(j == 3))
                        nc.vector.tensor_copy(
                            osb[0][:, ob * NBW:(ob + 1) * NBW], ops[:])
                    return mk

                def out_dma(nt, osb):
                    def mk():
                        # alternate queues: 16x1MB on one queue serializes
                        # the tail
                        eng = nc.gpsimd if nt % 2 == 0 else nc.sync
                        eng.dma_start(
                            out=OUT[nt * 128:(nt + 1) * 128, :], in_=osb[0][:])
                    return mk

                for i in range(4):
                    nt = b * 4 + i
                    osb = []

                    def alloc(osb=osb):
                        osb.append(outp.tile([128, D], F32, tag="osb",
                                             name="osb"))
                    thunks.append((0, alloc, None))
                    for ob in range(4):
                        thunks.append((1010, ob_unit(nt, ob, osb), None))
                    thunks.append((0, out_dma(nt, osb), None))
                return thunks

            # ---- norm: ctx / softmax-denominator, into ctxT ---------------
            def emit_pair_norm(c_a, c_b, g, q0):
                # Critical prefix: 4 plain copies move the ctx rows and the
                # denominator rows (psum partition 64, from the ones column
                # of va) out of PSUM so the two c banks free immediately.
                # Everything after runs async: broadcast both denominators
                # into one psum bank (col groups 0/64), 1/x as exp(-ln x)
                # on ACT (same table set as the attention exp; DVE
                # reciprocal costs ~3.3us per call), then normalize. Head
                # B's rows reach partitions 64:128 via a sbuf-to-sbuf DMA
                # (engines cannot shift partitions).
                lrowA = smp.tile([65, NBW], F32R, tag="lrA", name="lrA")
                nc.vector.tensor_copy(lrowA[DH:DH + 1, :], c_a[DH:DH + 1, :])
                lrowB = smp.tile([65, NBW], F32R, tag="lrB", name="lrB")
                nc.vector.tensor_copy(lrowB[DH:DH + 1, :], c_b[DH:DH + 1, :])
                cuA = smp.tile([DH, NBW], F32, tag="cuA", name="cuA")
                nc.vector.tensor_copy(cuA[:], c_a[0:DH, :])
                cuB = smp.tile([DH, NBW], F32, tag="cuB", name="cuB")
                nc.vector.tensor_copy(cuB[:], c_b[0:DH, :])
                for lrow, cu, par in ((lrowA, cuA, 0), (lrowB, cuB, 1)):
                    rb_ps = msp.tile([DH, NBW], F32, tag="ms", name="rbps")
                    nc.tensor.matmul(rb_ps[:], ones_r[DH:DH + 1, 0:DH],
                                     lrow[DH:DH + 1, :], start=True, stop=True)
                    lg = smp.tile([DH, NBW], F32, tag=f"lg{par}", name="lg")
                    nc.scalar.activation(lg[:], rb_ps[:], LN)
                    rb = smp.tile([DH, NBW], F32, tag=f"rb{par}", name="rb")
                    nc.scalar.activation(rb[:], lg[:], EXP, scale=-1.0)
                    if par == 0:
                        nc.vector.tensor_mul(ctxT[g][0:DH, q0:q0 + NBW],
                                             cu[:], rb[:])
                    else:
                        tmp = smp.tile([DH, NBW], BF16, tag="ctmp",
                                       name="ctmp")
                        nc.vector.tensor_mul(tmp[:], cu[:], rb[:])
                        nc.sync.dma_start(
                            out=ctxT[g][DH:2 * DH, q0:q0 + NBW], in_=tmp[:])

            # ---- attention for one q-block, with filler interleave --------
            def attention(b):
                q0 = b * NBW
                n_kb = 4 * b + 4
                for g in range(4):
                    c_a = cpp.tile([DH + 1, NBW], F32, tag="c", name="ca")
                    c_b = cpp.tile([DH + 1, NBW], F32, tag="c", name="cb")
                    pend = None
                    for kb in range(n_kb):
                        m0 = kb * 128
                        diag = kb >= 4 * b
                        off = max(0, m0 - q0)
                        T = scp.tile([128, 2 * NBW], F32, tag="sc", name="T")
                        # scores: kv0 rows 0:64 / kv1 rows 64:128 run as
                        # concurrent PE row-groups. Both heads skip cols
                        # < off (fully-masked queries); the exp then runs
                        # per written span (one op when off == 0).
                        nc.tensor.matmul(T[:, off:NBW],
                                         kt[0:64, m0:m0 + 128],
                                         qt[g][0:64, q0 + off:q0 + NBW],
                                         start=True, stop=not diag,
                                         skip_group_check=True)
                        nc.tensor.matmul(T[:, NBW + off:2 * NBW],
                                         kt[64:128, m0:m0 + 128],
                                         qt[g][64:128, q0 + off:q0 + NBW],
                                         start=True, stop=not diag,
                                         skip_group_check=True)
                        if diag:
                            nc.tensor.matmul(T[:, off:off + 128],
                                             ident[:], maskneg[:],
                                             start=False, stop=True,
                                             skip_group_check=True)
                            nc.tensor.matmul(T[:, NBW + off:NBW + off + 128],
                                             ident[:], maskneg[:],
                                             start=False, stop=True,
                                             skip_group_check=True)
                        w = NBW - off
                        p = psbp.tile([128, 2 * NBW], BF16, tag="p", name="p")
                        if off == 0:
                            nc.scalar.activation(p[:], T[:], EXP,
                                                 scale=float(scale))
                            clock["act"] += (2 * NBW + 172) / 1.2 + 120
                        else:
                            nc.scalar.activation(p[:, off:NBW],
                                                 T[:, off:NBW], EXP,
                                                 scale=float(scale))
                            nc.scalar.activation(p[:, NBW + off:2 * NBW],
                                                 T[:, NBW + off:2 * NBW], EXP,
                                                 scale=float(scale))
                            clock["act"] += 2 * ((w + 172) / 1.2 + 120)
                        clock["pe"] += 3 * (w / 2.4 + 45)
                        if diag:
                            clock["pe"] += 2 * (128 / 2.4 + 45)
                        if pend is not None:
                            pend()
                        emit_filler_budget(allow_lo=(b == NB - 1))

                        def _ctx(kb=kb, off=off, p=p, c_a=c_a, c_b=c_b):
                            nc.tensor.matmul(
                                c_a[:, off:NBW],
                                va[kb][:, 0:DH + 1],
                                p[:, off:NBW],
                                start=(kb == 0), stop=(kb == n_kb - 1),
                                skip_group_check=True)
                            nc.tensor.matmul(
                                c_b[:, off:NBW],
                                va[kb][:, DH + 1:2 * DH + 2],
                                p[:, NBW + off:2 * NBW],
                                start=(kb == 0), stop=(kb == n_kb - 1),
                                skip_group_check=True)
                        pend = _ctx
                    pend()
                    emit_pair_norm(c_a, c_b, g, q0)
                    clock["act"] += 2 * ((NBW + 172) / 1.2 + 120)
                    clock["pe"] += 2 * (NBW / 2.4 + 45)
                    emit_filler_budget(allow_lo=(b == NB - 1))

            # ---- main schedule -------------------------------------------
            emit_xs_dma(0)
            for _cost, th, _key in proj_thunks(0):
                th()
            emit_xs_dma(1)
            for b in range(NB):
                if b + 1 < NB:
                    filler_hi.extend(proj_thunks(b + 1))
                if b + 2 < NB:
                    filler_hi.append(
                        (0, lambda b=b: emit_xs_dma(b + 2), (b + 2, -1)))
                if b >= 1:
                    filler_lo.extend(outproj_thunks(b - 1))
                attention(b)
                drain_hi()
                # stage boundary: queues sync up during the pure-PE drain
                clock["act"] = clock["pe"] = max(clock["act"], clock["pe"])
            # Tail: flush leftover lo first (its ctxT inputs are ready), then
            # the last block's out-projection; bank alternation keeps the PE
            # dense so the HAM gate can recover full clock.
            for _cost, fn, _key in filler_lo:
                fn()
            for _cost, fn, _key in outproj_thunks(NB - 1):
                fn()

    nc.compile()
    return nc


def kernel(x, Wq, Wk, Wv, Wo, bo):
    x = np.asarray(x, dtype=np.float32)
    Wq = np.asarray(Wq, dtype=np.float32)
    Wk = np.asarray(Wk, dtype=np.float32)
    Wv = np.asarray(Wv, dtype=np.float32)
    Wo = np.asarray(Wo, dtype=np.float32)
    bo = np.asarray(bo, dtype=np.float32)

    if "nc" not in _CACHED:
        _CACHED["nc"] = _build()
    nc = _CACHED["nc"]

    bf = ml_dtypes.bfloat16
    in_maps = []
    for c in range(8):
        b, t = c // 4, c % 4
        xTc = np.ascontiguousarray(x[b].T).astype(bf)
        # q slab g holds [kv-head 2t head g | kv-head 2t+1 head g]
        qcols = []
        for g in range(4):
            for kvl in range(KVPC):
                h = (2 * t + kvl) * G + g
                qcols.append(Wq[:, h * DH:(h + 1) * DH])
        wq_c = np.ascontiguousarray(np.concatenate(qcols, axis=1)).astype(bf)
        wk_c = np.ascontiguousarray(Wk[:, t * 128:(t + 1) * 128]).astype(bf)
        wv_c = np.ascontiguousarray(Wv[:, t * 128:(t + 1) * 128]).astype(bf)
        # Wo rows must follow the ctxT head-pair layout: slab g holds
        # [head (kv 2t, g) | head (kv 2t+1, g)]
        wrows = []
        for g in range(4):
            for kvl in range(KVPC):
                h = (2 * t + kvl) * G + g
                wrows.append(Wo[h * DH:(h + 1) * DH, :])
        wo_c = np.ascontiguousarray(np.concatenate(wrows, axis=0)).astype(bf)
        in_maps.append({"xT": xTc, "Wq": wq_c, "Wk": wk_c, "Wv": wv_c,
                        "Wo": wo_c})

    trace = bool(int(os.environ.get("GQA_TRACE", "0")))
    kwargs = {}
    if trace:
        import tempfile
        td = os.environ.get("GQA_TRACE_DIR") or tempfile.mkdtemp(prefix="gqa_")
        kwargs = dict(trace=True, tmpdir=td)
    res = run_bass_kernel_spmd(nc, in_maps, list(range(8)), **kwargs)
    _CACHED["last_result"] = res

    out = np.empty((B, N, D), dtype=np.float32)
    for b in range(B):
        acc = res.results[4 * b]["out"].astype(np.float32)
        for t in range(1, 4):
            acc = acc + res.results[4 * b + t]["out"]
        out[b] = acc + bo[None, :]
    return out
